# revision 39
# baseline (speedup 1.0000x reference)
"""AttentionWithRoPE on 8 trn2 NeuronCores.

Sharding (tensor-parallel over heads x data-parallel over batch):
  core c -> batch b = c // 4, head group g = c % 4 (heads [4g, 4g+4)).
Each core computes q/k/v projections for its 4 heads (columns
[512g, 512g+512) of Wq/Wk/Wv), causal attention with RoPE, and the
partial o_proj contribution  attn_out_local @ Wo[512g:512g+512, :].
The host gather sums the 4 partials per batch (row-parallel linear).

v2: bf16 on-chip everywhere (hs, weights, qT/kT/v, exp); f32 PSUM and
f32 output. All four weight blocks + cos/sin fully SBUF-resident and
loaded once; hs streamed per 512-column block; qT resident (no DRAM
spill). RoPE rotate-half via SBUF->SBUF DMA on bf16 tiles. Softmax
denominator l via ones-column matmul; 1/l broadcast across partitions
with gpsimd(Pool) partition_broadcast; PSUM evictions on Pool.
"""

import sys

for _p in ("/opt/trn_rl_repo", "/root/.axon_site/_ro/trn_rl_repo"):
    if _p not in sys.path:
        sys.path.insert(0, _p)

import ml_dtypes
import numpy as np

import concourse.bass as bass
import concourse.tile as tile
from concourse import bacc, mybir
from concourse.bass_utils import run_bass_kernel_spmd

f32 = mybir.dt.float32
bf16 = mybir.dt.bfloat16
EXP = mybir.ActivationFunctionType.Exp
COPY = mybir.ActivationFunctionType.Copy

B = 2
S = 2048
E = 2048
D = 128
HL = 4          # local heads per core
EL = HL * D     # 512, local projection width
NB = S // 512   # 4 query/key 512-blocks
EC = E // 128   # 16 contraction chunks
SCALE = float(1.0 / np.sqrt(D))

_CACHE = {}
LAST_EXEC_NS = None


def _build(repeat=1):
    from contextlib import ExitStack

    nc = bacc.Bacc("TRN2", target_bir_lowering=False, debug=False, num_devices=8)

    HST = nc.dram_tensor("hsT", [E, S], bf16, kind="ExternalInput")
    WQ = nc.dram_tensor("wq", [E, EL], bf16, kind="ExternalInput")
    WK = nc.dram_tensor("wk", [E, EL], bf16, kind="ExternalInput")
    WV = nc.dram_tensor("wv", [E, EL], bf16, kind="ExternalInput")
    WO = nc.dram_tensor("wo", [EL, E], bf16, kind="ExternalInput")
    COS = nc.dram_tensor("cosT", [D, S], bf16, kind="ExternalInput")
    SIN = nc.dram_tensor("sinTs", [D, S], bf16, kind="ExternalInput")  # sign-folded
    MSK = nc.dram_tensor("masks", [128, 4, 512], bf16, kind="ExternalInput")
    OUT = nc.dram_tensor("out", [S, E], f32, kind="ExternalOutput")

    with tile.TileContext(nc) as tc, nc.allow_low_precision("bf16 compute by design"):
        with ExitStack() as octx:
            # kernel-lifetime residents, per-partition KB:
            #   wq/wk/wv 16 each, wo 16, kT/qT/v 16 each, cos/sin 8, masks 4
            res = octx.enter_context(tc.tile_pool(name="res", bufs=1))
            wq_sb = res.tile([128, EC, EL], bf16, tag="wq")
            wk_sb = res.tile([128, EC, EL], bf16, tag="wk")
            wv_sb = res.tile([128, EC, EL], bf16, tag="wv")
            wo_sb = res.tile([128, HL, E], bf16, tag="wo")
            kT = [res.tile([128, S], bf16, tag=f"kT{h}", name=f"kT{h}") for h in range(HL)]
            qT = [res.tile([128, S], bf16, tag=f"qT{h}", name=f"qT{h}") for h in range(HL)]
            v_sb = res.tile([128, NB * 4, EL], bf16, tag="v")
            cos_sb = res.tile([128, S], bf16, tag="cos")
            sin_sb = res.tile([128, S], bf16, tag="sin")
            masks = res.tile([128, 4, 512], bf16, tag="masks")
            # spread the startup loads across engine DGE queues so they run
            # in parallel; j=0 needs wv (then wq/wk) as early as possible
            nc.sync.dma_start(wv_sb[:], WV[:].rearrange("(c p) m -> p c m", p=128))
            nc.sync.dma_start(wq_sb[:], WQ[:].rearrange("(c p) m -> p c m", p=128))
            nc.gpsimd.dma_start(wk_sb[:], WK[:].rearrange("(c p) m -> p c m", p=128))
            nc.gpsimd.dma_start(cos_sb[:], COS[:])
            nc.gpsimd.dma_start(sin_sb[:], SIN[:])
            nc.gpsimd.dma_start(masks[:], MSK[:])
            nc.gpsimd.dma_start(wo_sb[:], WO[:].rearrange("(c p) m -> p c m", p=128))

            for _rep in range(repeat):
                # ---- phase 1: qT, kT (RoPE'd) and v, all SBUF-resident ----
                with ExitStack() as ctx:
                    hsp = ctx.enter_context(tc.tile_pool(name="hs1", bufs=3))
                    tmp = ctx.enter_context(tc.tile_pool(name="tmp1", bufs=3))
                    pps = ctx.enter_context(tc.tile_pool(name="pps1", bufs=3, space="PSUM"))
                    vps = ctx.enter_context(tc.tile_pool(name="vps1", bufs=4, space="PSUM"))

                    hs_pre = {}

                    def load_hs(j, eng):
                        t = hsp.tile([128, EC, 512], bf16, tag="hscol",
                                     name=f"hs{_rep}_{j}")
                        eng.dma_start(
                            t[:],
                            HST[:, j * 512:(j + 1) * 512].rearrange(
                                "(c p) s -> p c s", p=128
                            ),
                        )
                        return t

                    # prefetch the first two hs blocks on the ACT queue, in
                    # parallel with the weight loads above
                    hs_pre[0] = load_hs(0, nc.scalar)
                    hs_pre[1] = load_hs(1, nc.scalar)

                    def rope_evict(dst, ps, cos_t, sin_t):
                        # dst = raw*cosT + rot(raw)*sinT_signed
                        raw = tmp.tile([128, 512], bf16, tag="qkraw")
                        nc.scalar.activation(raw[:], ps[:], COPY)
                        rot = tmp.tile([128, 512], bf16, tag="qkrot")
                        nc.sync.dma_start(rot[0:64, :], raw[64:128, :])
                        nc.sync.dma_start(rot[64:128, :], raw[0:64, :])
                        t1 = tmp.tile([128, 512], bf16, tag="ropet1")
                        nc.vector.tensor_mul(t1[:], raw[:], cos_t)
                        nc.vector.tensor_mul(dst, rot[:], sin_t)
                        nc.vector.tensor_add(dst, dst, t1[:])

                    for j in range(NB):
                        sl = slice(j * 512, (j + 1) * 512)
                        hs_t = hs_pre.pop(j) if j in hs_pre else load_hs(j, nc.sync)
                        # v: 4 s-subtile psums accumulate over e
                        vp = [
                            vps.tile([128, EL], f32, tag="vps", name=f"vp{_rep}_{j}_{i}")
                            for i in range(4)
                        ]
                        for e in range(EC):
                            for i in range(4):
                                nc.tensor.matmul(
                                    vp[i][:],
                                    hs_t[:, e, i * 128:(i + 1) * 128],
                                    wv_sb[:, e, :],
                                    start=(e == 0),
                                    stop=(e == EC - 1),
                                )
                        for i in range(4):
                            nc.scalar.activation(v_sb[:, j * 4 + i, :], vp[i][:], COPY)

                        for h in range(HL):
                            ps = pps.tile([128, 512], f32, tag="qkps")
                            for e in range(EC):
                                nc.tensor.matmul(
                                    ps[:],
                                    wq_sb[:, e, h * 128:(h + 1) * 128],
                                    hs_t[:, e, :],
                                    start=(e == 0),
                                    stop=(e == EC - 1),
                                )
                            rope_evict(qT[h][:, sl], ps, cos_sb[:, sl], sin_sb[:, sl])

                            ps2 = pps.tile([128, 512], f32, tag="qkps")
                            for e in range(EC):
                                nc.tensor.matmul(
                                    ps2[:],
                                    wk_sb[:, e, h * 128:(h + 1) * 128],
                                    hs_t[:, e, :],
                                    start=(e == 0),
                                    stop=(e == EC - 1),
                                )
                            rope_evict(kT[h][:, sl], ps2, cos_sb[:, sl], sin_sb[:, sl])

                # ---- phase 2: attention + o_proj ----
                with ExitStack() as ctx:
                    sbp = ctx.enter_context(tc.tile_pool(name="sb2", bufs=6))
                    onp = ctx.enter_context(tc.tile_pool(name="on2", bufs=8))
                    bcp = ctx.enter_context(tc.tile_pool(name="bc2", bufs=2))
                    lap = ctx.enter_context(tc.tile_pool(name="la2", bufs=4))
                    orp = ctx.enter_context(tc.tile_pool(name="or2", bufs=2))
                    scp = ctx.enter_context(tc.tile_pool(name="scps", bufs=4, space="PSUM"))
                    avp = ctx.enter_context(tc.tile_pool(name="avps", bufs=2, space="PSUM"))
                    opp = ctx.enter_context(tc.tile_pool(name="opps", bufs=2, space="PSUM"))

                    class OProj:
                        """One output row-block of o_proj, emitted in four
                        4-matmul subgroups so PE work interleaves finely with
                        the score/exp/AV stream."""

                        def __init__(self, jj, i, o_n):
                            self.jj, self.i, self.o_n = jj, i, o_n
                            self.n = 0
                            self.orow = orp.tile([128, E], f32, tag="orow",
                                                 name=f"orow{_rep}_{jj}_{i}")

                        def emit_subgroup(self):
                            # one n-pair, h-major so each o_norm lhsT is
                            # loaded once for two rhs columns of wo
                            if self.n >= 4:
                                return
                            i, jj = self.i, self.jj
                            ns = (self.n, self.n + 1)
                            ops = [
                                opp.tile([128, 512], f32, tag="op",
                                         name=f"op{_rep}_{jj}_{i}_{n}")
                                for n in ns
                            ]
                            for h in range(HL):
                                lhs = self.o_n[h][:, i * 128:(i + 1) * 128]
                                for t, n in enumerate(ns):
                                    nc.tensor.matmul(
                                        ops[t][:],
                                        lhs,
                                        wo_sb[:, h, n * 512:(n + 1) * 512],
                                        start=(h == 0),
                                        stop=(h == HL - 1),
                                    )
                            for t, n in enumerate(ns):
                                nc.vector.tensor_copy(
                                    self.orow[:, n * 512:(n + 1) * 512], ops[t][:]
                                )
                            self.n += 2
                            if self.n == 4:
                                nc.sync.dma_start(
                                    OUT[jj * 512 + i * 128:jj * 512 + (i + 1) * 128, :],
                                    self.orow[:],
                                )

                        def finish(self):
                            while self.n < 4:
                                self.emit_subgroup()

                    norm_ctr = [0]

                    def emit_norm(av_ps, lacc, o_norm, h):
                        # l = allreduce_over_partitions(lacc0 + lacc1); the
                        # whole chain runs on Pool + DVE, no PE involvement
                        lsum = bcp.tile([128, 512], f32, tag="lsum")
                        nc.gpsimd.tensor_add(lsum[:], lacc[0][:], lacc[1][:])
                        lred = bcp.tile([128, 512], f32, tag="lred")
                        nc.gpsimd.partition_all_reduce(
                            lred[:], lsum[:], channels=128,
                            reduce_op=bass.bass_isa.ReduceOp.add,
                        )
                        bc_sb = bcp.tile([128, 512], f32, tag="bcsb")
                        nc.vector.reciprocal(bc_sb[:], lred[:])
                        norm_ctr[0] += 1
                        on = onp.tile([128, 512], bf16, tag="onorm",
                                      name=f"on{_rep}_{norm_ctr[0]}_{h}")
                        nc.vector.tensor_mul(on[:], av_ps[:], bc_sb[:])
                        o_norm[h] = on

                    o_prev = None
                    for j in range(NB):
                        sl = slice(j * 512, (j + 1) * 512)
                        nkb = 4 * j + 4
                        o_norm = [None] * HL
                        deferred = None
                        for h in range(HL):
                            pending = OProj(j - 1, h, o_prev) if o_prev else None
                            qt = qT[h][:, sl]
                            av_ps = avp.tile([128, 512], f32, tag="av")
                            # softmax denominator: bf16 partial sums on DVE
                            # (two interleaved accumulators), partition-reduced
                            # by two ones-matmuls at the end
                            lacc = [
                                lap.tile([128, 512], bf16, tag="lacc",
                                         name=f"lacc{_rep}_{j}_{h}_{kk}")
                                for kk in range(2)
                            ]
                            for kb in range(nkb):
                                sc_ps = scp.tile([128, 512], f32, tag="sc")
                                nc.tensor.matmul(
                                    sc_ps[:],
                                    kT[h][:, kb * 128:(kb + 1) * 128],
                                    qt,
                                    start=True,
                                    stop=True,
                                )
                                ex = sbp.tile([128, 512], bf16, tag="expT")
                                nc.scalar.activation(ex[:], sc_ps[:], EXP, scale=SCALE)
                                m = kb - 4 * j
                                if m >= 0:  # diagonal block: causal mask
                                    nc.vector.tensor_mul(ex[:], ex[:], masks[:, m, :])
                                nc.tensor.matmul(
                                    av_ps[:],
                                    v_sb[:, kb, h * 128:(h + 1) * 128],
                                    ex[:],
                                    start=(kb == 0),
                                    stop=(kb == nkb - 1),
                                )
                                if kb < 2:
                                    nc.vector.tensor_copy(lacc[kb][:], ex[:])
                                else:
                                    nc.vector.tensor_add(
                                        lacc[kb % 2][:], lacc[kb % 2][:], ex[:]
                                    )
                                if pending is not None and kb >= 1:
                                    pending.emit_subgroup()
                                if deferred is not None and kb == 2:
                                    emit_norm(*deferred)
                                    deferred = None
                            if pending is not None:
                                pending.finish()
                            if deferred is not None:
                                emit_norm(*deferred)
                            # defer this head's normalization into the next
                            # head's kb stream (software pipelining)
                            deferred = (av_ps, lacc, o_norm, h)
                        emit_norm(*deferred)
                        o_prev = o_norm

                    for i in range(4):
                        OProj(NB - 1, i, o_prev).finish()

    nc.compile()
    return nc


def _get_nc(repeat=1):
    key = ("nc", repeat)
    if key not in _CACHE:
        _CACHE[key] = _build(repeat=repeat)
    return _CACHE[key]


def _make_masks():
    sk = np.arange(128)[:, None]
    sq = np.arange(512)[None, :]
    m = np.stack([(sq >= sk + 128 * mm) for mm in range(4)], axis=1)
    return m.astype(ml_dtypes.bfloat16)


def _prepare_in_maps(hidden_states, cos, sin, Wq, Wk, Wv, Wo):
    bf = ml_dtypes.bfloat16
    hidden_states = np.asarray(hidden_states, dtype=np.float32)
    cos = np.asarray(cos, dtype=np.float32)
    sin = np.asarray(sin, dtype=np.float32)

    masks = _make_masks()
    in_maps = []
    hsT = [np.ascontiguousarray(hidden_states[b].T).astype(bf) for b in range(B)]
    cosT = [np.ascontiguousarray(cos[b].T).astype(bf) for b in range(B)]
    sinTs = []
    for b in range(B):
        s = np.ascontiguousarray(sin[b].T)
        s[:64] *= -1.0
        sinTs.append(s.astype(bf))
    Wq = np.asarray(Wq, dtype=np.float32).astype(bf)
    Wk = np.asarray(Wk, dtype=np.float32).astype(bf)
    Wv = np.asarray(Wv, dtype=np.float32).astype(bf)
    Wo = np.asarray(Wo, dtype=np.float32).astype(bf)
    for c in range(8):
        b, g = c // 4, c % 4
        cols = slice(512 * g, 512 * (g + 1))
        in_maps.append({
            "hsT": hsT[b],
            "wq": np.ascontiguousarray(Wq[:, cols]),
            "wk": np.ascontiguousarray(Wk[:, cols]),
            "wv": np.ascontiguousarray(Wv[:, cols]),
            "wo": np.ascontiguousarray(Wo[cols, :]),
            "cosT": cosT[b],
            "sinTs": sinTs[b],
            "masks": masks,
        })
    return in_maps


def kernel(hidden_states, cos, sin, Wq, Wk, Wv, Wo):
    nc = _get_nc()
    in_maps = _prepare_in_maps(hidden_states, cos, sin, Wq, Wk, Wv, Wo)
    res = run_bass_kernel_spmd(nc, in_maps, core_ids=list(range(8)))
    global LAST_EXEC_NS
    if res.exec_time_ns is not None:
        LAST_EXEC_NS = res.exec_time_ns
    out = np.empty((B, S, E), dtype=np.float32)
    for b in range(B):
        acc = res.results[4 * b]["out"].astype(np.float32)
        for g in range(1, 4):
            acc = acc + res.results[4 * b + g]["out"]
        out[b] = acc
    return out
